# revision 5
# baseline (speedup 1.0000x reference)
"""CrossAttention kernel for 8 Trainium2 NeuronCores.

Sharding: core c handles batch b = c // 2 and S1-half = c % 2 (512 query rows).
Each core computes full K/V for its batch (duplicated across the pair sharing a
batch — cheaper than exchanging them), runs attention + output projection +
LayerNorm stats for its 512 rows, and returns a partial pooled sum. The host
combines the two partials per batch and applies gamma/beta.

Device layouts are feature-major (x^T, W^T prepared on host), so every matmul
contracts over the partition dim with no on-device transposes:
  QT = WqT.T @ X1T   KT = WkT.T @ X2T    V = X2T.T @ WvT  (natural layout)
  ST_h = KT_h.T @ QT_h  -> exp via ACT with the mask folded into the bias
  O'_h = V_aug.T @ expT (V has a ones column -> row 64 is the softmax denom)
  YT = WoT.T @ OT  (+ bias bo + x_cls via per-partition bias)
  LN stats via ones-matmuls; pooled output via a weighted free-dim reduce.
"""

from contextlib import ExitStack

import numpy as np
import ml_dtypes

try:
    import concourse.bass as bass  # noqa: F401
except ImportError:
    import sys

    for p in ("/opt/trn_rl_repo", "/root/.axon_site/_ro/trn_rl_repo"):
        sys.path.insert(0, p)
    import concourse.bass as bass  # noqa: F401

import concourse.tile as tile
from concourse import bacc, mybir
from concourse.bass_utils import run_bass_kernel_spmd

B, S1, S2 = 4, 1024, 1024
IN1 = IN2 = 1024
H, KD, VD = 16, 64, 64
S1C = S1 // 2  # rows per core
P = 128
NKB = IN1 // P  # contraction blocks
NJB = IN2 // P
NS2B = S2 // P
LN_EPS = 1e-5

BF16 = mybir.dt.bfloat16
F32 = mybir.dt.float32
AF = mybir.ActivationFunctionType


def _emit(nc, tc, io, ctx):
    x1t, x2t, wqt, wkt, wvt, wot, maskbias, bco, acc_out, csum_out = io

    const = ctx.enter_context(tc.tile_pool(name="const", bufs=1))
    wpool = ctx.enter_context(tc.tile_pool(name="w", bufs=2))
    qkv = ctx.enter_context(tc.tile_pool(name="qkv", bufs=1))
    epool = ctx.enter_context(tc.tile_pool(name="exp", bufs=2))
    hpool = ctx.enter_context(tc.tile_pool(name="head", bufs=2))
    lnpool = ctx.enter_context(tc.tile_pool(name="ln", bufs=1))
    scratch = ctx.enter_context(tc.tile_pool(name="scratch", bufs=2))
    ps = ctx.enter_context(tc.tile_pool(name="ps", bufs=3, space="PSUM"))
    ps_o = ctx.enter_context(tc.tile_pool(name="ps_o", bufs=2, space="PSUM"))
    ps_st = ctx.enter_context(tc.tile_pool(name="ps_st", bufs=1, space="PSUM"))

    # ---- constants / activations in ----
    x1t_sb = const.tile([P, NKB, S1C], BF16)
    nc.sync.dma_start(x1t_sb[:], x1t.rearrange("(kb p) s -> p kb s", p=P))
    x2t_sb = const.tile([P, NKB, S2], BF16)
    nc.sync.dma_start(x2t_sb[:], x2t.rearrange("(kb p) s -> p kb s", p=P))
    mb_sb = const.tile([P, NS2B], F32)
    nc.sync.dma_start(mb_sb[:], maskbias.rearrange("(kb p) -> p kb", p=P))
    bco_sb = const.tile([P, NJB], F32)
    nc.sync.dma_start(bco_sb[:], bco.rearrange("(jb p) -> p jb", p=P))
    ones_sb = const.tile([P, 1], BF16)
    nc.vector.memset(ones_sb[:], 1.0)

    # ---- Q projection: qt_sb[p, dqb, s] with dq = dqb*128 + p ----
    wq_sb = wpool.tile([P, NKB, IN1], BF16, tag="w")
    nc.sync.dma_start(wq_sb[:], wqt.rearrange("(kb p) d -> p kb d", p=P))
    qt_sb = qkv.tile([P, NKB, S1C], BF16)
    for dqb in range(NKB):
        pt = ps.tile([P, S1C], F32)
        for kb in range(NKB):
            nc.tensor.matmul(
                pt[:],
                wq_sb[:, kb, dqb * P : (dqb + 1) * P],
                x1t_sb[:, kb, :],
                start=(kb == 0),
                stop=(kb == NKB - 1),
            )
        nc.scalar.copy(qt_sb[:, dqb, :], pt[:])

    # ---- K projection: kt_sb[p, dkb, t] ----
    wk_sb = wpool.tile([P, NKB, IN2], BF16, tag="w")
    nc.sync.dma_start(wk_sb[:], wkt.rearrange("(kb p) d -> p kb d", p=P))
    kt_sb = qkv.tile([P, NKB, S2], BF16)
    for dkb in range(NKB):
        for s2h in range(2):
            pt = ps.tile([P, 512], F32)
            for kb in range(NKB):
                nc.tensor.matmul(
                    pt[:],
                    wk_sb[:, kb, dkb * P : (dkb + 1) * P],
                    x2t_sb[:, kb, s2h * 512 : (s2h + 1) * 512],
                    start=(kb == 0),
                    stop=(kb == NKB - 1),
                )
            nc.scalar.copy(kt_sb[:, dkb, s2h * 512 : (s2h + 1) * 512], pt[:])

    # ---- V projection, natural layout + ones column: v_sb[p, s2b, h, 0:64|64] ----
    wv_sb = wpool.tile([P, NKB, IN2], BF16, tag="w")
    nc.sync.dma_start(wv_sb[:], wvt.rearrange("(kb p) d -> p kb d", p=P))
    v_sb = qkv.tile([P, NS2B, H, VD + 1], BF16)
    nc.vector.memset(v_sb[:, :, :, VD : VD + 1], 1.0)
    for s2b in range(NS2B):
        for dvh in range(2):
            pt = ps.tile([P, 512], F32)
            for kb in range(NKB):
                nc.tensor.matmul(
                    pt[:],
                    x2t_sb[:, kb, s2b * P : (s2b + 1) * P],
                    wv_sb[:, kb, dvh * 512 : (dvh + 1) * 512],
                    start=(kb == 0),
                    stop=(kb == NKB - 1),
                )
            nc.scalar.copy(
                v_sb[:, s2b, dvh * 8 : (dvh + 1) * 8, 0:VD],
                pt[:].rearrange("p (h d) -> p h d", d=VD),
            )

    # ---- attention per head ----
    ot_sb = qkv.tile([P, NKB, S1C], BF16)
    for h in range(H):
        po = (h % 2) * 64  # partition offset of this head inside its 128-block
        blk = h // 2
        et = epool.tile([P, NS2B, S1C], BF16, tag="et")
        for s2b in range(NS2B):
            pt = ps.tile([P, S1C], F32)
            nc.tensor.matmul(
                pt[:],
                kt_sb[po : po + 64, blk, s2b * P : (s2b + 1) * P],
                qt_sb[po : po + 64, blk, :],
                start=True,
                stop=True,
            )
            nc.scalar.activation(
                et[:, s2b, :],
                pt[:],
                AF.Exp,
                bias=mb_sb[:, s2b : s2b + 1],
                scale=1.0 / (KD**0.5),
            )
        pv = ps_o.tile([VD + 1, S1C], F32)
        for s2b in range(NS2B):
            nc.tensor.matmul(
                pv[:],
                v_sb[:, s2b, h, :],
                et[:, s2b, :],
                start=(s2b == 0),
                stop=(s2b == NS2B - 1),
            )
        r1 = hpool.tile([1, S1C], F32, tag="r1")
        nc.vector.reciprocal(r1[:], pv[VD : VD + 1, :])
        rb = hpool.tile([64, S1C], F32, tag="rb")
        nc.gpsimd.partition_broadcast(rb[:], r1[:])
        nc.vector.tensor_mul(ot_sb[po : po + 64, blk, :], pv[0:VD, :], rb[:])

    # ---- output projection + residual/bias, LN stats ----
    wo_sb = wpool.tile([P, NKB, IN2], BF16, tag="w")
    nc.sync.dma_start(wo_sb[:], wot.rearrange("(kb p) d -> p kb d", p=P))
    yt_sb = qkv.tile([P, NJB, S1C], BF16)
    pmu = ps_st.tile([1, S1C], F32, tag="pmu")
    psq = ps_st.tile([1, S1C], F32, tag="psq")
    for jb in range(NJB):
        pt = ps.tile([P, S1C], F32)
        for kb in range(NKB):
            nc.tensor.matmul(
                pt[:],
                wo_sb[:, kb, jb * P : (jb + 1) * P],
                ot_sb[:, kb, :],
                start=(kb == 0),
                stop=(kb == NKB - 1),
            )
        nc.scalar.activation(
            yt_sb[:, jb, :], pt[:], AF.Identity, bias=bco_sb[:, jb : jb + 1]
        )
        ysq = scratch.tile([P, S1C], BF16, tag="ysq")
        nc.scalar.square(ysq[:], yt_sb[:, jb, :])
        nc.tensor.matmul(
            pmu[:],
            ones_sb[:],
            yt_sb[:, jb, :],
            start=(jb == 0),
            stop=(jb == NJB - 1),
        )
        nc.tensor.matmul(
            psq[:],
            ones_sb[:],
            ysq[:],
            start=(jb == 0),
            stop=(jb == NJB - 1),
        )

    # ---- LN scalars: mu, var, rstd (all [1, S1C]) ----
    mu_sb = lnpool.tile([1, S1C], F32, tag="mu")
    nc.scalar.activation(mu_sb[:], pmu[:], AF.Copy, scale=1.0 / IN2)
    ex2_sb = lnpool.tile([1, S1C], F32, tag="ex2")
    nc.scalar.activation(ex2_sb[:], psq[:], AF.Copy, scale=1.0 / IN2)
    musq_sb = lnpool.tile([1, S1C], F32, tag="musq")
    nc.scalar.square(musq_sb[:], mu_sb[:])
    var_sb = lnpool.tile([1, S1C], F32, tag="var")
    nc.vector.tensor_sub(var_sb[:], ex2_sb[:], musq_sb[:])
    eps_sb = lnpool.tile([1, 1], F32, tag="eps")
    nc.vector.memset(eps_sb[:], LN_EPS)
    sd_sb = lnpool.tile([1, S1C], F32, tag="sd")
    nc.scalar.activation(sd_sb[:], var_sb[:], AF.Sqrt, bias=eps_sb[:])
    rstd_sb = lnpool.tile([1, S1C], F32, tag="rstd")
    nc.vector.reciprocal(rstd_sb[:], sd_sb[:])

    # csum = sum_s mu[s] * rstd[s]
    c_sb = lnpool.tile([1, 1], F32, tag="c")
    t1 = lnpool.tile([1, S1C], F32, tag="t1")
    nc.vector.tensor_mul(t1[:], mu_sb[:], rstd_sb[:])
    nc.vector.reduce_sum(c_sb[:], t1[:], axis=mybir.AxisListType.X)

    # acc[j] = sum_s y[j, s] * rstd[s]
    rstdb_sb = lnpool.tile([P, S1C], F32, tag="rstdb")
    nc.gpsimd.partition_broadcast(rstdb_sb[:], rstd_sb[:])
    acc_sb = lnpool.tile([P, NJB], F32, tag="acc")
    for jb in range(NJB):
        t2 = scratch.tile([P, S1C], F32, tag="t2")
        nc.vector.tensor_mul(t2[:], yt_sb[:, jb, :], rstdb_sb[:])
        nc.vector.reduce_sum(
            acc_sb[:, jb : jb + 1], t2[:], axis=mybir.AxisListType.X
        )

    nc.sync.dma_start(acc_out.rearrange("(jb p) -> p jb", p=P), acc_sb[:])
    nc.sync.dma_start(csum_out[:], c_sb[:])


_CACHED = None


def _build():
    global _CACHED
    if _CACHED is not None:
        return _CACHED
    nc = bacc.Bacc("TRN2", target_bir_lowering=False, debug=False, num_devices=8)
    io = (
        nc.dram_tensor("x1t", [IN1, S1C], BF16, kind="ExternalInput").ap(),
        nc.dram_tensor("x2t", [IN2, S2], BF16, kind="ExternalInput").ap(),
        nc.dram_tensor("wqt", [IN1, KD * H], BF16, kind="ExternalInput").ap(),
        nc.dram_tensor("wkt", [IN2, KD * H], BF16, kind="ExternalInput").ap(),
        nc.dram_tensor("wvt", [IN2, VD * H], BF16, kind="ExternalInput").ap(),
        nc.dram_tensor("wot", [VD * H, IN2], BF16, kind="ExternalInput").ap(),
        nc.dram_tensor("maskbias", [S2], F32, kind="ExternalInput").ap(),
        nc.dram_tensor("bco", [IN2], F32, kind="ExternalInput").ap(),
        nc.dram_tensor("acc", [IN2], F32, kind="ExternalOutput").ap(),
        nc.dram_tensor("csum", [1, 1], F32, kind="ExternalOutput").ap(),
    )
    with tile.TileContext(nc) as tc:
        with ExitStack() as ctx:
            _emit(nc, tc, io, ctx)
    nc.compile()
    _CACHED = nc
    return nc


def make_in_maps(x1, x2, mask, Wq, Wk, Wv, Wo, bo, gamma, beta):
    bf = ml_dtypes.bfloat16
    wqt = np.ascontiguousarray(Wq.T).astype(bf)
    wkt = np.ascontiguousarray(Wk.T).astype(bf)
    wvt = np.ascontiguousarray(Wv.T).astype(bf)
    wot = np.ascontiguousarray(Wo.T).astype(bf)
    in_maps = []
    for c in range(8):
        b, half = c // 2, c % 2
        x1t = np.ascontiguousarray(x1[b, half * S1C : (half + 1) * S1C, :].T).astype(bf)
        x2t = np.ascontiguousarray(x2[b].T).astype(bf)
        maskbias = ((mask[b, 0, 0].astype(np.float32) - 1.0) * 10000.0).astype(
            np.float32
        )
        bco = (bo + x2[b, 0]).astype(np.float32)
        in_maps.append(
            {
                "x1t": x1t,
                "x2t": x2t,
                "wqt": wqt,
                "wkt": wkt,
                "wvt": wvt,
                "wot": wot,
                "maskbias": maskbias,
                "bco": bco,
            }
        )
    return in_maps


def run(inputs, trace=False, **kw):
    nc = _build()
    in_maps = make_in_maps(**inputs)
    return run_bass_kernel_spmd(nc, in_maps, list(range(8)), trace=trace, **kw)


def kernel(**inputs):
    res = run(inputs)
    gamma = inputs["gamma"].astype(np.float32)
    beta = inputs["beta"].astype(np.float32)
    out = np.zeros((B, IN2), np.float32)
    for b in range(B):
        r0, r1 = res.results[2 * b], res.results[2 * b + 1]
        acc = r0["acc"] + r1["acc"]
        c = float(r0["csum"].reshape(-1)[0] + r1["csum"].reshape(-1)[0])
        out[b] = gamma * (acc - c) / np.float32(IN2) + beta
    return out
